# revision 11
# baseline (speedup 1.0000x reference)
"""Trainium2 Bass kernel: per-point 3x3 Gaussian covariance from quaternion + log_scale.

cov = R diag(exp(log_scale)) R^T  with R built from the normalized quaternion.

Layout strategy (per core):
  - N is sharded evenly across 8 cores (padded so each shard is 128*R points).
  - Within a core, points are viewed as [128 partitions, R rows]: partition p
    owns R consecutive points.  Tiles process F points per partition at a time;
    all DMAs are per-partition contiguous (16F/12F/36F bytes per partition).
  - Normalization is folded algebraically: with n2 = |q|^2 and inv2 = 2/n2,
    A..D = q*inv2 give doubled-normalized products directly (no sqrt needed).
  - M = R * exp(0.5*log_scale), cov = M M^T (exactly symmetric, 6 unique entries).
"""

import os
import numpy as np

import concourse.bass as bass
import concourse.bacc as bacc
import concourse.mybir as mybir
from concourse.tile import TileContext
from concourse.bass_utils import run_bass_kernel_spmd

AF = mybir.ActivationFunctionType
FP32 = mybir.dt.float32

N_CORES = 8
N_FULL = 4_000_000
P = 128
R = 3908                      # rows per partition per core; 128*3908*8 = 4_001_792 >= N
NPC = P * R                   # points per core (padded)
F = int(os.environ.get("KERNEL_F", "256"))  # points per partition per tile

SQRT_HALF = 0.7071067811865476

_built = {}


def _build():
    key = F
    if key in _built:
        return _built[key]

    nc = bacc.Bacc("TRN2", target_bir_lowering=False, debug=False, num_devices=N_CORES)
    q = nc.dram_tensor("q", [NPC, 4], FP32, kind="ExternalInput")
    ls = nc.dram_tensor("ls", [NPC, 3], FP32, kind="ExternalInput")
    cov = nc.dram_tensor("cov", [NPC, 3, 3], FP32, kind="ExternalOutput")

    qv = q.ap().rearrange("(p r) c -> p (r c)", p=P)       # [128, 4R]
    lsv = ls.ap().rearrange("(p r) c -> p (r c)", p=P)     # [128, 3R]
    ov = cov.ap().rearrange("(p r) i k -> p (r i k)", p=P)  # [128, 9R]

    with TileContext(nc) as tc:
        with (
            tc.tile_pool(name="io", bufs=3) as io,
            tc.tile_pool(name="big", bufs=2) as big,
            tc.tile_pool(name="wk", bufs=2) as wk,
        ):
            t0 = 0
            while t0 < R:
                f = min(F, R - t0)
                _tile_body(nc, io, big, wk, qv, lsv, ov, t0, f)
                t0 += f

    nc.compile()
    _built[key] = nc
    return nc


def _split_waits(nc, max_waits=2):
    """Walrus codegen caps the number of sync-wait commands one instruction can
    carry (the kernel-tail drain ends up waiting on 10 sems).  Move excess
    waits onto same-engine NoOps inserted right before the instruction."""
    for func in nc.m.functions:
        for blk in func.blocks:
            idx = 0
            while idx < len(blk.instructions):
                inst = blk.instructions[idx]
                cap = 1 if isinstance(inst, (mybir.InstDMACopy, mybir.InstDrain)) else max_waits
                si = getattr(inst, "sync_info", None)
                if si is not None and si.on_wait and len(si.on_wait) > cap:
                    waits = list(si.on_wait)
                    keep = waits[:cap]
                    rest = waits[cap:]
                    si.on_wait = keep
                    pos = idx
                    while rest:
                        chunk, rest = rest[:max_waits], rest[max_waits:]
                        nop = mybir.InstNoOp(
                            name=nc.get_next_instruction_name(), ins=[], outs=[]
                        )
                        nop.engine = inst.engine
                        nop.sync_info = mybir.SyncInfo(on_wait=chunk, on_update=[])
                        blk.instructions.insert(pos, nop)
                        idx += 1
                idx += 1


def _tile_body(nc, io, big, wk, qv, lsv, ov, t0, f):
    cnt = [0]

    def w(tag=None):
        cnt[0] += 1
        tag = tag or f"w{cnt[0]}"
        return wk.tile([P, f], FP32, tag=tag, name=f"{tag}_t{t0}")

    qt = io.tile([P, 4 * f], FP32, tag="qt", name=f"qt{t0}")
    lst = io.tile([P, 3 * f], FP32, tag="lst", name=f"lst{t0}")
    nc.sync.dma_start(out=qt, in_=qv[:, 4 * t0:4 * (t0 + f)])
    nc.sync.dma_start(out=lst, in_=lsv[:, 3 * t0:3 * (t0 + f)])

    qc = qt.rearrange("p (f c) -> p f c", c=4)

    # ---- n2/2 = (a^2+b^2+c^2+d^2)/2, inv2 = 2/n2 -------------------------
    sq4 = big.tile([P, 4 * f], FP32, tag="sq4", name=f"sq4_{t0}")
    nc.scalar.activation(sq4, qt, AF.Square, scale=SQRT_HALF)  # x^2/2
    sqc = sq4.rearrange("p (f c) -> p f c", c=4)
    u = w(); v = w(); n2h = w(); inv2 = w()
    nc.vector.tensor_add(u, sqc[:, :, 0], sqc[:, :, 1])
    nc.vector.tensor_add(v, sqc[:, :, 2], sqc[:, :, 3])
    nc.vector.tensor_add(n2h, u, v)
    lnv = w()
    nc.scalar.activation(lnv, n2h, AF.Ln)
    nc.scalar.activation(inv2, lnv, AF.Exp, scale=-1.0)        # 1/n2h = 2/|q|^2

    # ---- doubled normalized components and products ----------------------
    A = w(); B = w(); C = w(); D = w()
    nc.vector.tensor_mul(A, inv2, qc[:, :, 0])
    nc.vector.tensor_mul(B, inv2, qc[:, :, 1])
    nc.vector.tensor_mul(C, inv2, qc[:, :, 2])
    nc.vector.tensor_mul(D, inv2, qc[:, :, 3])

    Ab = w(); Ac = w(); Ad = w()
    Bb = w(); Bc = w(); Bd = w()
    Cc = w(); Cd = w(); Dd = w()
    nc.vector.tensor_mul(Ab, A, qc[:, :, 1])
    nc.vector.tensor_mul(Ac, A, qc[:, :, 2])
    nc.vector.tensor_mul(Ad, A, qc[:, :, 3])
    nc.vector.tensor_mul(Bb, B, qc[:, :, 1])
    nc.vector.tensor_mul(Bc, B, qc[:, :, 2])
    nc.vector.tensor_mul(Bd, B, qc[:, :, 3])
    nc.vector.tensor_mul(Cc, C, qc[:, :, 2])
    nc.vector.tensor_mul(Cd, C, qc[:, :, 3])
    nc.vector.tensor_mul(Dd, D, qc[:, :, 3])

    # ---- rotation matrix entries ----------------------------------------
    t_0 = w(); t_1 = w(); t_2 = w()
    nc.vector.tensor_add(t_0, Cc, Dd)
    nc.vector.tensor_add(t_1, Bb, Dd)
    nc.vector.tensor_add(t_2, Bb, Cc)
    r00 = w(); r11 = w(); r22 = w()
    nc.scalar.activation(r00, t_0, AF.Identity, bias=1.0, scale=-1.0)
    nc.scalar.activation(r11, t_1, AF.Identity, bias=1.0, scale=-1.0)
    nc.scalar.activation(r22, t_2, AF.Identity, bias=1.0, scale=-1.0)
    r01 = w(); r10 = w(); r02 = w(); r20 = w(); r12 = w(); r21 = w()
    nc.vector.tensor_sub(r01, Bc, Ad)
    nc.vector.tensor_add(r10, Bc, Ad)
    nc.vector.tensor_add(r02, Bd, Ac)
    nc.vector.tensor_sub(r20, Bd, Ac)
    nc.vector.tensor_sub(r12, Cd, Ab)
    nc.vector.tensor_add(r21, Cd, Ab)

    # ---- M = R * sqrt(scale) --------------------------------------------
    sh = big.tile([P, 3 * f], FP32, tag="sh", name=f"sh_{t0}")
    nc.scalar.activation(sh, lst, AF.Exp, scale=0.5)           # exp(ls/2)
    shc = sh.rearrange("p (f c) -> p f c", c=3)
    Rm = [[r00, r01, r02], [r10, r11, r12], [r20, r21, r22]]
    M = [[None] * 3 for _ in range(3)]
    for i in range(3):
        for j in range(3):
            M[i][j] = w()
            nc.vector.tensor_mul(M[i][j], Rm[i][j], shc[:, :, j])

    # ---- cov = M M^T (6 unique entries) ---------------------------------
    covt = {}
    for (i, k) in [(0, 0), (0, 1), (0, 2), (1, 1), (1, 2), (2, 2)]:
        g = w(); h = w()
        nc.vector.tensor_mul(g, M[i][0], M[k][0])
        nc.vector.tensor_mul(h, M[i][1], M[k][1])
        g2 = w()
        nc.vector.tensor_add(g2, g, h)
        h2 = w()
        nc.vector.tensor_mul(h2, M[i][2], M[k][2])
        cik = w()
        nc.vector.tensor_add(cik, g2, h2)
        covt[(i, k)] = cik

    # ---- interleave into [.., 9] output layout ---------------------------
    ot = io.tile([P, 9 * f], FP32, tag="ot", name=f"ot_{t0}")
    otv = ot.rearrange("p (f e) -> p f e", e=9)
    order = [(0, 0), (0, 1), (0, 2), (0, 1), (1, 1), (1, 2), (0, 2), (1, 2), (2, 2)]
    for e, ik in enumerate(order):
        nc.scalar.copy(out=otv[:, :, e], in_=covt[ik])

    nc.sync.dma_start(out=ov[:, 9 * t0:9 * (t0 + f)], in_=ot)


def _pad_and_shard(quaternion, log_scale):
    n = quaternion.shape[0]
    pad = N_CORES * NPC - n
    if pad:
        qpad = np.tile(np.array([1, 0, 0, 0], np.float32), (pad, 1))
        lpad = np.zeros((pad, 3), np.float32)
        quaternion = np.concatenate([quaternion, qpad], axis=0)
        log_scale = np.concatenate([log_scale, lpad], axis=0)
    in_maps = []
    for i in range(N_CORES):
        sl = slice(i * NPC, (i + 1) * NPC)
        in_maps.append({
            "q": np.ascontiguousarray(quaternion[sl]),
            "ls": np.ascontiguousarray(log_scale[sl]),
        })
    return in_maps


def kernel_with_stats(quaternion, log_scale, trace=False):
    quaternion = np.asarray(quaternion, dtype=np.float32)
    log_scale = np.asarray(log_scale, dtype=np.float32)
    n = quaternion.shape[0]
    nc = _build()
    in_maps = _pad_and_shard(quaternion, log_scale)
    res = run_bass_kernel_spmd(nc, in_maps, core_ids=list(range(N_CORES)), trace=trace)
    out = np.concatenate([r["cov"] for r in res.results], axis=0)[:n]
    return out, res


def kernel(quaternion, log_scale):
    out, _ = kernel_with_stats(quaternion, log_scale, trace=False)
    return out


# revision 14
# speedup vs baseline: 1.7141x; 1.7141x over previous
"""Trainium2 Bass kernel: per-point 3x3 Gaussian covariance from quaternion + log_scale.

cov = R diag(exp(log_scale)) R^T  with R built from the normalized quaternion.

Layout (per core): points sharded [128 partitions, R rows]; tiles of F points
per partition; all DMAs per-partition contiguous.  Normalization folded via
inv2 = 2/|q|^2 (computed fp32 as exp(-ln(n2/2))); the multiply-heavy chain
(products -> R -> M -> Gram) runs in bf16 with contiguous step-1 operands so
VectorE hits its 2x perf mode; ScalarE does the strided deinterleave/cast,
squares, exp/ln, and output interleave.
"""

import os
import numpy as np

import concourse.bass as bass
import concourse.bacc as bacc
import concourse.mybir as mybir
from concourse.tile import TileContext
from concourse.bass_utils import run_bass_kernel_spmd

AF = mybir.ActivationFunctionType
FP32 = mybir.dt.float32
BF16 = mybir.dt.bfloat16

N_CORES = 8
N_FULL = 4_000_000
P = 128
R = 3908                      # rows per partition per core; 128*3908*8 = 4_001_792 >= N
NPC = P * R                   # points per core (padded)
F = int(os.environ.get("KERNEL_F", "512"))  # points per partition per tile

SQRT_HALF = 0.7071067811865476

_built = {}


def _build():
    key = F
    if key in _built:
        return _built[key]

    nc = bacc.Bacc("TRN2", target_bir_lowering=False, debug=False, num_devices=N_CORES)
    q = nc.dram_tensor("q", [NPC, 4], FP32, kind="ExternalInput")
    ls = nc.dram_tensor("ls", [NPC, 3], FP32, kind="ExternalInput")
    cov = nc.dram_tensor("cov", [NPC, 3, 3], FP32, kind="ExternalOutput")

    qv = q.ap().rearrange("(p r) c -> p (r c)", p=P)       # [128, 4R]
    lsv = ls.ap().rearrange("(p r) c -> p (r c)", p=P)     # [128, 3R]
    ov = cov.ap().rearrange("(p r) i k -> p (r i k)", p=P)  # [128, 9R]

    with TileContext(nc) as tc:
        with (
            tc.tile_pool(name="io", bufs=3) as io,
            tc.tile_pool(name="otp", bufs=2) as ot_pool,
            tc.tile_pool(name="big", bufs=2) as big,
            tc.tile_pool(name="wk", bufs=2) as wk,
        ):
            t0 = 0
            while t0 < R:
                f = min(F, R - t0)
                _tile_body(nc, io, ot_pool, big, wk, qv, lsv, ov, t0, f)
                t0 += f

    nc.compile()
    _built[key] = nc
    return nc


def _tile_body(nc, io, ot_pool, big, wk, qv, lsv, ov, t0, f):
    cnt = [0]

    def w(dt=BF16, tag=None):
        cnt[0] += 1
        tag = tag or f"w{cnt[0]}"
        return wk.tile([P, f], dt, tag=tag, name=f"{tag}_t{t0}_{cnt[0]}")

    qt = io.tile([P, 4 * f], FP32, tag="qt", name=f"qt{t0}")
    lst = io.tile([P, 3 * f], FP32, tag="lst", name=f"lst{t0}")
    nc.sync.dma_start(out=qt, in_=qv[:, 4 * t0:4 * (t0 + f)])
    nc.sync.dma_start(out=lst, in_=lsv[:, 3 * t0:3 * (t0 + f)])

    qc = qt.rearrange("p (f c) -> p f c", c=4)
    lsc = lst.rearrange("p (f c) -> p f c", c=3)

    # ---- fp32 path: n2/2 and inv2 = 2/|q|^2 = exp(-ln(n2/2)) -------------
    sq4 = big.tile([P, 4 * f], FP32, tag="sq4", name=f"sq4_{t0}")
    nc.scalar.activation(sq4, qt, AF.Square, scale=SQRT_HALF)  # x^2/2
    sqc = sq4.rearrange("p (f c) -> p f c", c=4)
    u = w(FP32, tag="fu"); v = w(FP32, tag="fv"); n2h = w(FP32, tag="fn2h")
    lnv = w(FP32, tag="fu"); inv2 = w(FP32, tag="fv")
    nc.vector.tensor_add(u, sqc[:, :, 0], sqc[:, :, 1])
    nc.vector.tensor_add(v, sqc[:, :, 2], sqc[:, :, 3])
    nc.vector.tensor_add(n2h, u, v)
    nc.scalar.activation(lnv, n2h, AF.Ln)
    nc.scalar.activation(inv2, lnv, AF.Exp, scale=-1.0)

    # ---- deinterleave + cast to bf16 (ScalarE, strided reads) ------------
    a_ = w(); b_ = w(); c_ = w(); d_ = w(); ivb = w()
    nc.scalar.copy(out=a_, in_=qc[:, :, 0])
    nc.scalar.copy(out=b_, in_=qc[:, :, 1])
    nc.scalar.copy(out=c_, in_=qc[:, :, 2])
    nc.scalar.copy(out=d_, in_=qc[:, :, 3])
    nc.scalar.copy(out=ivb, in_=inv2)

    # ---- bf16 chain: A..D, products (VectorE 2x mode) --------------------
    A = w(); B = w(); C = w(); D = w()
    nc.vector.tensor_mul(A, ivb, a_)
    nc.vector.tensor_mul(B, ivb, b_)
    nc.vector.tensor_mul(C, ivb, c_)
    nc.vector.tensor_mul(D, ivb, d_)

    Ab = w(); Ac = w(); Ad = w()
    Bb = w(); Bc = w(); Bd = w()
    Cc = w(); Cd = w(); Dd = w()
    nc.vector.tensor_mul(Ab, A, b_)
    nc.vector.tensor_mul(Ac, A, c_)
    nc.vector.tensor_mul(Ad, A, d_)
    nc.vector.tensor_mul(Bb, B, b_)
    nc.vector.tensor_mul(Bc, B, c_)
    nc.vector.tensor_mul(Bd, B, d_)
    nc.vector.tensor_mul(Cc, C, c_)
    nc.vector.tensor_mul(Cd, C, d_)
    nc.vector.tensor_mul(Dd, D, d_)

    # ---- rotation matrix entries (bf16) ----------------------------------
    t_0 = w(); t_1 = w(); t_2 = w()
    nc.vector.tensor_add(t_0, Cc, Dd)
    nc.vector.tensor_add(t_1, Bb, Dd)
    nc.vector.tensor_add(t_2, Bb, Cc)
    r00 = w(); r11 = w(); r22 = w()
    nc.scalar.activation(r00, t_0, AF.Identity, bias=1.0, scale=-1.0)
    nc.scalar.activation(r11, t_1, AF.Identity, bias=1.0, scale=-1.0)
    nc.scalar.activation(r22, t_2, AF.Identity, bias=1.0, scale=-1.0)
    r01 = w(); r10 = w(); r02 = w(); r20 = w(); r12 = w(); r21 = w()
    nc.vector.tensor_sub(r01, Bc, Ad)
    nc.vector.tensor_add(r10, Bc, Ad)
    nc.vector.tensor_add(r02, Bd, Ac)
    nc.vector.tensor_sub(r20, Bd, Ac)
    nc.vector.tensor_sub(r12, Cd, Ab)
    nc.vector.tensor_add(r21, Cd, Ab)

    # ---- sqrt(scale) per column (ScalarE, bf16 contiguous out) -----------
    sh = [w(), w(), w()]
    for j in range(3):
        nc.scalar.activation(sh[j], lsc[:, :, j], AF.Exp, scale=0.5)

    Rm = [[r00, r01, r02], [r10, r11, r12], [r20, r21, r22]]
    M = [[None] * 3 for _ in range(3)]
    for i in range(3):
        for j in range(3):
            M[i][j] = w(tag=f"pm{i}{j}")
            nc.vector.tensor_mul(M[i][j], Rm[i][j], sh[j])

    # ---- cov = M M^T; diag entries write straight into the out tile ------
    ot = ot_pool.tile([P, 9 * f], FP32, tag="ot", name=f"ot_{t0}")
    otv = ot.rearrange("p (f e) -> p f e", e=9)
    offd = {}
    for (i, k) in [(0, 0), (0, 1), (0, 2), (1, 1), (1, 2), (2, 2)]:
        g = w(tag="gg"); h = w(tag="gh"); g2 = w(tag="gg2"); h2 = w(tag="gh2")
        nc.vector.tensor_mul(g, M[i][0], M[k][0])
        nc.vector.tensor_mul(h, M[i][1], M[k][1])
        nc.vector.tensor_add(g2, g, h)
        nc.vector.tensor_mul(h2, M[i][2], M[k][2])
        if i == k:
            nc.vector.tensor_add(otv[:, :, 3 * i + k], g2, h2)  # fp32 strided out
        else:
            cik = w(tag=f"cov{i}{k}")
            nc.vector.tensor_add(cik, g2, h2)
            offd[(i, k)] = cik

    # off-diagonals + symmetric duplicates via ScalarE copies (cast to fp32)
    for (i, k), cik in offd.items():
        nc.scalar.copy(out=otv[:, :, 3 * i + k], in_=cik)
        nc.scalar.copy(out=otv[:, :, 3 * k + i], in_=cik)

    nc.sync.dma_start(out=ov[:, 9 * t0:9 * (t0 + f)], in_=ot)


def _pad_and_shard(quaternion, log_scale):
    n = quaternion.shape[0]
    pad = N_CORES * NPC - n
    if pad:
        qpad = np.tile(np.array([1, 0, 0, 0], np.float32), (pad, 1))
        lpad = np.zeros((pad, 3), np.float32)
        quaternion = np.concatenate([quaternion, qpad], axis=0)
        log_scale = np.concatenate([log_scale, lpad], axis=0)
    in_maps = []
    for i in range(N_CORES):
        sl = slice(i * NPC, (i + 1) * NPC)
        in_maps.append({
            "q": np.ascontiguousarray(quaternion[sl]),
            "ls": np.ascontiguousarray(log_scale[sl]),
        })
    return in_maps


def kernel_with_stats(quaternion, log_scale, trace=False):
    quaternion = np.asarray(quaternion, dtype=np.float32)
    log_scale = np.asarray(log_scale, dtype=np.float32)
    n = quaternion.shape[0]
    nc = _build()
    in_maps = _pad_and_shard(quaternion, log_scale)
    res = run_bass_kernel_spmd(nc, in_maps, core_ids=list(range(N_CORES)), trace=trace)
    out = np.concatenate([r["cov"] for r in res.results], axis=0)[:n]
    return out, res


def kernel(quaternion, log_scale):
    out, _ = kernel_with_stats(quaternion, log_scale, trace=False)
    return out
